# revision 11
# baseline (speedup 1.0000x reference)
"""Multi-head cross-attention TRN2 Bass kernel, sharded over 8 NeuronCores.

Problem (nn_MultiHeadCrossAttention): B=2, Sq=1024, Skv=4096 (text+image+
audio+video), hidden=1024, heads=16, head_dim=64, out=4096.

Sharding: core c = 4*b + hg handles batch b and head-group hg (4 heads).
Schedule (v2 — single software pipeline keeping PE busy end to end):
  P1:  Q proj (fft streamed 2-tiles/DMA, sync queue) interleaved with
       K proj (kv blocks, gpsimd queue).  V proj for kv block 0 at the
       tail of P1; blocks 1-7 are projected inside head 0's attention
       window (PE slack while exp runs on ACT).
  C:   per head: scores -> exp(s/16) -> PV (sw-pipelined one kv tile
       behind).  Normalization of head h is deferred into head h+1's
       window so the PE never stalls on the DVE reciprocal.  Phase-D
       partial for head-pair 0 is interleaved into heads 2-3.  bv is
       folded into the host-side output bias (bo' = bo + Wo @ bv).
  D:   pair-1 partial as a short tail; head 3's reciprocal runs on the
       ACT engine as exp(-ln(den)) (same activation table set as exp).
Host sums the 8 per-(batch, pair) partials and adds bo'.
"""

import numpy as np

import bass_rust
import concourse.bass as bass
import concourse.mybir as mybir
import concourse.tile as tile
from concourse.bass_utils import run_bass_kernel_spmd
from concourse.vector_clock import ScopedClock

# ---------------------------------------------------------------------------
# Workarounds for walrus per-instruction sync-wait caps (this walrus build
# rejects instructions carrying more waits than the ISA slot count; Tile's
# sem assignment can attach more). Split excess waits onto single-wait nops.
# ---------------------------------------------------------------------------
import re as _re

_VC_RE = _re.compile(r"VectorClock\(\[([0-9, ]*)\]\)")


def _vc_values(vc):
    m = _VC_RE.match(repr(vc))
    assert m, repr(vc)
    s = m.group(1).strip()
    return [int(x) for x in s.split(",")] if s else []


def _split_excess_waits(tc, ordered_instructions_by_block, max_waits=1):
    nc = tc.nc
    for _bb, insts in ordered_instructions_by_block.items():
        out = []
        for inst in insts:
            si = inst.sync_info
            waits = list(si.on_wait) if si and si.on_wait else []
            if len(waits) > max_waits:
                keep = waits[:max_waits]
                for w in waits[max_waits:]:
                    nop = mybir.InstNoOp(
                        name=nc.get_next_instruction_name(), ins=[], outs=[]
                    )
                    nop.engine = inst.engine
                    nop.sync_info = bass_rust.SyncInfo(on_wait=[w], on_update=[])
                    nc.register_instruction(nop)
                    out.append(nop)
                inst.sync_info = bass_rust.SyncInfo(
                    on_wait=keep, on_update=list(si.on_update or [])
                )
            out.append(inst)
        insts[:] = out


_orig_lower = tile.TileContext._lower_ordered_insts


def _lower_with_split(self, postordered_blocks):
    _split_excess_waits(self, postordered_blocks)
    return _orig_lower(self, postordered_blocks)


def _drain_and_barrier_split(self, tick_clock, wait_clock):
    vals = _vc_values(tick_clock.global_clock)
    for proc_idx, tick in enumerate(vals):
        if tick <= 0:
            continue
        single = [0] * len(vals)
        single[proc_idx] = tick
        nop_inst = self.nc.sync.nop(nofuse=True, hint=f"drain_wait_p{proc_idx}")
        wait_clock.add_sem_waits(
            nop_inst.ins, ScopedClock({None: bass_rust.VectorClock(single)})
        )
    self.nc.sync.drain()
    self.nc.all_engine_barrier()
    assert self.sems is not None
    popped = self.nc._tile_sem_poison_stack.pop()
    assert popped is self._sem_poison
    self.nc.clear_and_free_semaphores(list(self.sems.allocated().values()))
    self.nc.all_engine_barrier()


tile.TileContext._lower_ordered_insts = _lower_with_split
tile.TileContext._drain_and_barrier = _drain_and_barrier_split

# ---------------------------------------------------------------------------
# Problem constants (hardcoded per contract)
# ---------------------------------------------------------------------------
B = 2
SQ = 1024
SKV = 4096
HID = 1024
HEADS = 16
DH = 64
DOUT = 4096
NCORES = 8
HG = 4  # head-groups (cores per batch)
GHEADS = HEADS // HG  # heads per group = 4
GF = GHEADS * DH  # feature slice per group = 256
NPAIR = GHEADS // 2  # head pairs per group = 2

F32 = mybir.dt.float32
BF16 = mybir.dt.bfloat16
FP16 = mybir.dt.float16
DT_MM = BF16
NP_MM = "bfloat16"
Exp = mybir.ActivationFunctionType.Exp
Ln = mybir.ActivationFunctionType.Ln
MUL = mybir.AluOpType.mult
ADD = mybir.AluOpType.add

NKVT = SKV // 128  # 32 kv tiles
NKVB = 8  # kv blocks (512 wide)
NFT_Q = 4096 // 128  # 32 contraction tiles for Q proj
NFT_KV = HID // 128  # 8 contraction tiles for K/V proj
NSQH = SQ // 512  # 2 sq halves
NJT = DOUT // 128  # 32 output row tiles

_NC_CACHE = {}


def build():
    if "nc" in _NC_CACHE:
        return _NC_CACHE["nc"]
    nc = bass.Bass()

    fft = nc.declare_dram_parameter("fft", [4096, SQ], DT_MM, isOutput=False)
    kvt = nc.declare_dram_parameter("kvt", [NKVB, HID, 512], DT_MM, isOutput=False)
    wqt = nc.declare_dram_parameter("wqt", [4096, GF], DT_MM, isOutput=False)
    wkt = nc.declare_dram_parameter("wkt", [HID, GF], DT_MM, isOutput=False)
    wvt = nc.declare_dram_parameter("wvt", [HID, GF], DT_MM, isOutput=False)
    wot = nc.declare_dram_parameter("wot", [GF, DOUT], DT_MM, isOutput=False)
    bq = nc.declare_dram_parameter("bq", [128, NPAIR], F32, isOutput=False)
    bk = nc.declare_dram_parameter("bk", [128, NPAIR], F32, isOutput=False)
    outp = nc.declare_dram_parameter("outp", [NPAIR, DOUT, SQ], FP16, isOutput=True)

    with tile.TileContext(nc) as tc:
        with (
            tc.tile_pool(name="hold", bufs=1) as hold,
            tc.tile_pool(name="misc", bufs=1) as misc,
            tc.tile_pool(name="kvs", bufs=NKVB) as kvs,
        ):
            # ---- activation-table warm: force natural_log_exp set early ----
            warm = misc.tile([1, 64], F32, tag="warm")
            nc.vector.memset(warm[:], 1.0)
            warm2 = misc.tile([1, 64], F32, tag="warm2")
            nc.scalar.activation(warm2[:], warm[:], Ln)
            nc.scalar.activation(warm2[:], warm2[:], Exp)

            # ---- long-lived tiles ----
            wkt_r = hold.tile([128, NFT_KV, NPAIR, 128], DT_MM, tag="wkt")
            nc.sync.dma_start(
                out=wkt_r[:],
                in_=wkt.rearrange("(ft p) (pr d) -> p ft pr d", p=128, pr=NPAIR),
            )
            wvt_r = hold.tile([128, NFT_KV, GF], DT_MM, tag="wvt")
            nc.sync.dma_start(
                out=wvt_r[:], in_=wvt.rearrange("(ft p) d -> p ft d", p=128)
            )
            wot_r = hold.tile([128, NPAIR, DOUT], DT_MM, tag="wot")
            bq_t = misc.tile([128, NPAIR], F32, tag="bq")
            nc.sync.dma_start(out=bq_t[:], in_=bq[:])
            bk_t = misc.tile([128, NPAIR], F32, tag="bk")
            nc.sync.dma_start(out=bk_t[:], in_=bk[:])

            ones_f = misc.tile([128, GHEADS], F32, tag="ones_f")
            nc.vector.memset(ones_f[:], 1.0)
            ones_row = misc.tile([1, DH], DT_MM, tag="ones_row")
            nc.vector.tensor_copy(ones_row[:], ones_f[0:1, 0:1].broadcast_to([1, DH]))

            qt_r = hold.tile([128, GHEADS, SQ], DT_MM, tag="qt")
            kt_r = hold.tile([128, GHEADS, SKV], DT_MM, tag="kt")
            v_r = hold.tile([128, NKVT, GHEADS, 128], DT_MM, tag="v")
            att_r = hold.tile([128, NPAIR, SQ], DT_MM, tag="att")

            kv_blocks = [None] * NKVB

            def v_chunk(kvt_i, pool, tag):
                """Project V for one 128-wide kv tile (one kl chunk)."""
                kb, kl = divmod(kvt_i, 4)
                kv_t = kv_blocks[kb]
                v_ps = pool.tile([128, GF], F32, tag=tag,
                                 name=f"v_ps{kvt_i}")
                for ft in range(NFT_KV):
                    nc.tensor.matmul(
                        v_ps[:],
                        kv_t[:, ft, 128 * kl : 128 * (kl + 1)],
                        wvt_r[:, ft, :],
                        start=(ft == 0),
                        stop=(ft == NFT_KV - 1),
                    )
                nc.vector.tensor_copy(
                    v_r[:, kvt_i, :, 0:DH],
                    v_ps.rearrange("p (h d) -> p h d", h=GHEADS),
                )
                nc.vector.tensor_copy(v_r[:, kvt_i, :, DH : DH + 1], ones_f[:, :])

            # ================= Phase 1: Q + K projections =================
            with (
                nc.named_scope("phaseAB_proj"),
                tc.tile_pool(name="ffts", bufs=3) as ffts,
                tc.tile_pool(name="wqs", bufs=3) as wqs,
                tc.tile_pool(name="psA", bufs=4, space="PSUM") as psA,
                tc.tile_pool(name="psB", bufs=4, space="PSUM") as psB,
            ):
                qt_ps = [
                    psA.tile([128, 512], F32, tag="psA", name=f"qt_ps{i}")
                    for i in range(4)
                ]  # (pair, sqh)
                cur = {}

                def a_step(kt):
                    if kt % 2 == 0:
                        j = kt // 2
                        fft_t = ffts.tile([128, 2, SQ], DT_MM, tag="fft",
                                          name=f"fft{j}")
                        nc.sync.dma_start(
                            out=fft_t[:],
                            in_=fft[256 * j : 256 * (j + 1), :].rearrange(
                                "(two p) s -> p two s", two=2
                            ),
                        )
                        wq_t = wqs.tile([128, 2, NPAIR, 128], DT_MM, tag="wq",
                                        name=f"wq{j}")
                        nc.sync.dma_start(
                            out=wq_t[:],
                            in_=wqt[256 * j : 256 * (j + 1), :].rearrange(
                                "(two p) (pr d) -> p two pr d", two=2, pr=NPAIR
                            ),
                        )
                        cur["fft"], cur["wq"] = fft_t, wq_t
                    half = kt % 2
                    for pr in range(NPAIR):
                        for sh in range(NSQH):
                            nc.tensor.matmul(
                                qt_ps[pr * NSQH + sh][:],
                                cur["wq"][:, half, pr, :],
                                cur["fft"][:, half, 512 * sh : 512 * (sh + 1)],
                                start=(kt == 0),
                                stop=(kt == NFT_Q - 1),
                            )

                def k_block(kb):
                    kv_t = kvs.tile([128, NFT_KV, 512], DT_MM, tag="kv",
                                    name=f"kv{kb}")
                    kv_blocks[kb] = kv_t
                    nc.gpsimd.dma_start(
                        out=kv_t[:],
                        in_=kvt[kb].rearrange("(ft p) n -> p ft n", p=128),
                    )
                    kb_sl = slice(512 * kb, 512 * (kb + 1))
                    for pr in range(NPAIR):
                        kt_ps = psB.tile([128, 512], F32, tag="psB",
                                         name=f"kt_ps{kb}_{pr}")
                        for ft in range(NFT_KV):
                            nc.tensor.matmul(
                                kt_ps[:],
                                wkt_r[:, ft, pr, :],
                                kv_t[:, ft, :],
                                start=(ft == 0),
                                stop=(ft == NFT_KV - 1),
                            )
                        nc.vector.tensor_scalar(
                            kt_r[0:64, 2 * pr, kb_sl],
                            kt_ps[0:64, :],
                            bk_t[0:64, pr : pr + 1],
                            None,
                            ADD,
                        )
                        nc.vector.tensor_scalar(
                            kt_r[64:128, 2 * pr + 1, kb_sl],
                            kt_ps[64:128, :],
                            bk_t[64:128, pr : pr + 1],
                            None,
                            ADD,
                        )
                        nc.scalar.dma_start(
                            out=kt_r[64:128, 2 * pr, kb_sl],
                            in_=kt_r[0:64, 2 * pr, kb_sl],
                        )
                        nc.scalar.dma_start(
                            out=kt_r[0:64, 2 * pr + 1, kb_sl],
                            in_=kt_r[64:128, 2 * pr + 1, kb_sl],
                        )

                for kt in range(NFT_Q):
                    if kt % 4 == 0:
                        k_block(kt // 4)
                    a_step(kt)
                    if kt >= 28:
                        v_chunk(kt - 28, psB, "psB")  # kv block 0

                for pr in range(NPAIR):
                    for sh in range(NSQH):
                        sq_sl = slice(512 * sh, 512 * (sh + 1))
                        nc.vector.tensor_scalar(
                            qt_r[0:64, 2 * pr, sq_sl],
                            qt_ps[pr * NSQH + sh][0:64, :],
                            bq_t[0:64, pr : pr + 1],
                            None,
                            ADD,
                        )
                        nc.vector.tensor_scalar(
                            qt_r[64:128, 2 * pr + 1, sq_sl],
                            qt_ps[pr * NSQH + sh][64:128, :],
                            bq_t[64:128, pr : pr + 1],
                            None,
                            ADD,
                        )
                # duplicate halves so score matmuls contract K=128 (2x scores,
                # folded into the exp scale) -- keeps the PE fully row-active.
                for pr in range(NPAIR):
                    nc.scalar.dma_start(
                        out=qt_r[64:128, 2 * pr, :], in_=qt_r[0:64, 2 * pr, :]
                    )
                    nc.scalar.dma_start(
                        out=qt_r[0:64, 2 * pr + 1, :],
                        in_=qt_r[64:128, 2 * pr + 1, :],
                    )

            # ================= Phase C: attention =================
            # wot is only needed mid-phase-C; its DMA overlaps the start.
            nc.sync.dma_start(
                out=wot_r[:], in_=wot.rearrange("(pr p) j -> p pr j", p=128)
            )
            with (
                nc.named_scope("phaseC_attn"),
                tc.tile_pool(name="pp", bufs=3) as pp,
                tc.tile_pool(name="nrm", bufs=2) as nrm,
                tc.tile_pool(name="osb", bufs=3) as osb,
                tc.tile_pool(name="psS", bufs=2, space="PSUM") as psS,
                tc.tile_pool(name="psAtt", bufs=2, space="PSUM") as psAtt,
                tc.tile_pool(name="psX", bufs=2, space="PSUM") as psX,
            ):
                def emit_norm(hd, att_sb, den, sh, on_act):
                    pr, h = hd // 2, hd % 2
                    sq_sl = slice(512 * sh, 512 * (sh + 1))
                    rec = nrm.tile([1, 512], DT_MM, tag="rec",
                                   name=f"rec{hd}{sh}")
                    if on_act:
                        t1 = nrm.tile([1, 512], F32, tag="lnd",
                                      name=f"lnd{hd}{sh}")
                        nc.scalar.activation(t1[:], den[:, sh, :], Ln)
                        with nc.allow_low_precision(reason="softmax recip"):
                            nc.scalar.activation(rec[:], t1[:], Exp, scale=-1.0)
                    else:
                        with nc.allow_low_precision(reason="softmax recip"):
                            nc.vector.reciprocal(rec[:], den[:, sh, :])
                    rb = psX.tile([DH, 512], F32, tag="psX", name=f"rb{hd}{sh}")
                    nc.tensor.matmul(
                        rb[:], ones_row[0:1, :], rec[0:1, :],
                        start=True, stop=True,
                    )
                    with nc.allow_low_precision(reason="bf16 att store"):
                        nc.vector.tensor_tensor(
                            att_r[64 * h : 64 * (h + 1), pr, sq_sl],
                            att_sb[:, sh, :],
                            rb[:],
                            MUL,
                        )

                def _copy_osb(o_sb, sh, src, eng):
                    dst = o_sb[:, 512 * sh : 512 * (sh + 1)]
                    if eng == "scalar":
                        nc.scalar.activation(
                            dst, src, mybir.ActivationFunctionType.Copy
                        )
                    elif eng == "gpsimd":
                        nc.gpsimd.tensor_copy(dst, src)
                    else:
                        nc.vector.tensor_copy(dst, src)

                def d_jt(jt, pr, pool, tag, shape2, cp_engines):
                    j_sl = slice(128 * jt, 128 * (jt + 1))
                    o_sb = osb.tile([128, SQ], FP16, tag="osb",
                                    name=f"osb{pr}_{jt}")
                    if shape2:
                        o_ps = pool.tile([128, NSQH, 512], F32, tag=tag,
                                         name=f"o{pr}_{jt}")
                        for sh in range(NSQH):
                            nc.tensor.matmul(
                                o_ps[:, sh, :],
                                wot_r[:, pr, j_sl],
                                att_r[:, pr, 512 * sh : 512 * (sh + 1)],
                                start=True,
                                stop=True,
                            )
                        for sh in range(NSQH):
                            _copy_osb(o_sb, sh, o_ps[:, sh, :],
                                      cp_engines[sh % len(cp_engines)])
                    else:
                        for sh in range(NSQH):
                            o_ps = pool.tile([128, 512], F32, tag=tag,
                                             name=f"o{pr}_{jt}_{sh}")
                            nc.tensor.matmul(
                                o_ps[:],
                                wot_r[:, pr, j_sl],
                                att_r[:, pr, 512 * sh : 512 * (sh + 1)],
                                start=True,
                                stop=True,
                            )
                            _copy_osb(o_sb, sh, o_ps[:],
                                      cp_engines[sh % len(cp_engines)])
                    nc.sync.dma_start(out=outp[pr, j_sl, :], in_=o_sb[:])

                # D-pr0 slots: (head, kv) pairs during heads 2-3
                d0_slots = [(2, kv) for kv in range(6, 32)] + [
                    (3, kv) for kv in range(0, 6)
                ]
                d0_iter = iter(range(NJT))
                d0_map = {slot: jt for slot, jt in zip(d0_slots, d0_iter)}

                pending = []  # (hd, att_sb, den) awaiting norm emission

                for hd in range(GHEADS):
                    att_ps = [
                        psAtt.tile([128, 512], F32, tag="att",
                                   name=f"att{hd}_{sh}")
                        for sh in range(NSQH)
                    ]

                    def pv(kv, p, att_ps=att_ps, hd=hd):
                        for sh in range(NSQH):
                            nc.tensor.matmul(
                                att_ps[sh][:],
                                v_r[:, kv, hd, :],
                                p[:, sh, :],
                                start=(kv == 0),
                                stop=(kv == NKVT - 1),
                            )

                    pq = []  # pending (kv, p) awaiting PV
                    for kv in range(NKVT):
                        s_ps = psS.tile([128, NSQH, 512], F32, tag="s",
                                        name=f"s{hd}_{kv}")
                        kv_sl = slice(128 * kv, 128 * (kv + 1))
                        for sh in range(NSQH):
                            nc.tensor.matmul(
                                s_ps[:, sh, :],
                                kt_r[:, hd, kv_sl],
                                qt_r[:, hd, 512 * sh : 512 * (sh + 1)],
                                start=True,
                                stop=True,
                            )
                        p = pp.tile([128, NSQH, 512], DT_MM, tag="p",
                                    name=f"p{hd}_{kv}")
                        nc.scalar.activation(p[:], s_ps[:], Exp, scale=0.0625)
                        pq.append((kv, p))
                        if kv >= 1:
                            pv(*pq.pop(0))
                        # ---- interleaved extra PE work ----
                        if hd == 0 and kv <= 27:
                            v_chunk(kv + 4, psX, "psX")
                        if pending and kv in (2, 4):
                            ph, psb, pden = pending[0]
                            emit_norm(ph, psb, pden, 0 if kv == 2 else 1,
                                      on_act=False)
                            if kv == 4:
                                pending.pop(0)
                        jt = d0_map.get((hd, kv))
                        if jt is not None:
                            d_jt(jt, 0, psX, "psX", False, ("vector",))
                    pv(*pq.pop(0))

                    # free att psum banks: copy numerator + denominator to SBUF
                    att_sb = nrm.tile([DH, NSQH, 512], F32, tag="attsb",
                                      name=f"attsb{hd}")
                    den = nrm.tile([1, NSQH, 512], F32, tag="den",
                                   name=f"den{hd}")
                    for sh in range(NSQH):
                        nc.vector.tensor_copy(att_sb[:, sh, :],
                                              att_ps[sh][0:DH, :])
                        nc.vector.tensor_copy(den[:, sh, :],
                                              att_ps[sh][DH : DH + 1, :])
                    if hd < GHEADS - 1:
                        pending.append((hd, att_sb, den))
                    else:
                        # last head: reciprocal on ACT (exp stream is done)
                        for sh in range(NSQH):
                            emit_norm(hd, att_sb, den, sh, on_act=True)

                # ---- tail: D partial for pair 1 ----
                cp_cycle = (("vector", "scalar"), ("scalar", "vector"))
                for jt in range(NJT):
                    d_jt(jt, 1, psS, "s", True, cp_cycle[jt % 2])

    _NC_CACHE["nc"] = nc
    return nc


def _make_in_maps(inputs):
    ff = np.asarray(inputs["fused_features"], dtype=np.float32)
    kv_in = np.concatenate(
        [
            np.asarray(inputs["text"], dtype=np.float32),
            np.asarray(inputs["image"], dtype=np.float32),
            np.asarray(inputs["audio"], dtype=np.float32),
            np.asarray(inputs["video"], dtype=np.float32),
        ],
        axis=1,
    )
    Wq = np.asarray(inputs["Wq"], dtype=np.float32)
    Wk = np.asarray(inputs["Wk"], dtype=np.float32)
    Wv = np.asarray(inputs["Wv"], dtype=np.float32)
    Wo = np.asarray(inputs["Wo"], dtype=np.float32)
    bq = np.asarray(inputs["bq"], dtype=np.float32)
    bk = np.asarray(inputs["bk"], dtype=np.float32)

    import ml_dtypes

    np_mm = np.dtype(ml_dtypes.bfloat16) if NP_MM == "bfloat16" else np.float32
    ffT = [np.ascontiguousarray(ff[b].T.astype(np_mm)) for b in range(B)]
    kvT = [
        np.ascontiguousarray(
            kv_in[b].T.astype(np_mm).reshape(HID, NKVB, 512).transpose(1, 0, 2)
        )
        for b in range(B)
    ]
    WqT = np.ascontiguousarray(Wq.T.astype(np_mm))  # [4096, 1024]
    WkT = np.ascontiguousarray(Wk.T.astype(np_mm))  # [1024, 1024]
    WvT = np.ascontiguousarray(Wv.T.astype(np_mm))
    WoT = np.ascontiguousarray(Wo.T.astype(np_mm))  # [1024, 4096]

    in_maps = []
    for c in range(NCORES):
        b, hg = divmod(c, HG)
        fs = slice(GF * hg, GF * (hg + 1))
        in_maps.append(
            {
                "fft": ffT[b],
                "kvt": kvT[b],
                "wqt": np.ascontiguousarray(WqT[:, fs]),
                "wkt": np.ascontiguousarray(WkT[:, fs]),
                "wvt": np.ascontiguousarray(WvT[:, fs]),
                "wot": np.ascontiguousarray(WoT[fs, :]),
                "bq": np.ascontiguousarray(bq[fs].reshape(NPAIR, 128).T),
                "bk": np.ascontiguousarray(bk[fs].reshape(NPAIR, 128).T),
            }
        )
    return in_maps


def _assemble(results, bo, Wo, bv):
    out = np.zeros((B, SQ, DOUT), dtype=np.float32)
    for c in range(NCORES):
        b = c // HG
        partial = results[c]["outp"].astype(np.float32)  # [NPAIR, DOUT, SQ]
        out[b] += (partial[0] + partial[1]).T
    bo_prime = np.asarray(bo, dtype=np.float32) + np.asarray(
        Wo, dtype=np.float32
    ) @ np.asarray(bv, dtype=np.float32)
    out += bo_prime
    return out


def run_spmd(inputs, trace=False):
    nc = build()
    in_maps = _make_in_maps(inputs)
    r = run_bass_kernel_spmd(nc, in_maps, list(range(NCORES)), trace=trace)
    return _assemble(r.results, inputs["bo"], inputs["Wo"], inputs["bv"]), r


def kernel(**inputs) -> np.ndarray:
    out, _ = run_spmd(inputs, trace=False)
    return out


# revision 21
# speedup vs baseline: 1.0638x; 1.0638x over previous
"""Multi-head cross-attention TRN2 Bass kernel, sharded over 8 NeuronCores.

Problem (nn_MultiHeadCrossAttention): B=2, Sq=1024, Skv=4096 (text+image+
audio+video), hidden=1024, heads=16, head_dim=64, out=4096.

Sharding: core c = 4*b + hg handles batch b and head-group hg (4 heads).
Schedule (v2 — single software pipeline keeping PE busy end to end):
  P1:  Q proj (fft streamed 2-tiles/DMA, sync queue) interleaved with
       K proj (kv blocks, gpsimd queue).  V proj for kv block 0 at the
       tail of P1; blocks 1-7 are projected inside head 0's attention
       window (PE slack while exp runs on ACT).
  C:   per head: scores -> exp(s/16) -> PV (sw-pipelined one kv tile
       behind).  Normalization of head h is deferred into head h+1's
       window so the PE never stalls on the DVE reciprocal.  Phase-D
       partial for head-pair 0 is interleaved into heads 2-3.  bv is
       folded into the host-side output bias (bo' = bo + Wo @ bv).
  D:   pair-1 partial as a short tail; head 3's reciprocal runs on the
       ACT engine as exp(-ln(den)) (same activation table set as exp).
Host sums the 8 per-(batch, pair) partials and adds bo'.
"""

import numpy as np

import bass_rust
import concourse.bass as bass
import concourse.mybir as mybir
import concourse.tile as tile
from concourse.bass_utils import run_bass_kernel_spmd
from concourse.vector_clock import ScopedClock

# ---------------------------------------------------------------------------
# Workarounds for walrus per-instruction sync-wait caps (this walrus build
# rejects instructions carrying more waits than the ISA slot count; Tile's
# sem assignment can attach more). Split excess waits onto single-wait nops.
# ---------------------------------------------------------------------------
import re as _re

_VC_RE = _re.compile(r"VectorClock\(\[([0-9, ]*)\]\)")


def _vc_values(vc):
    m = _VC_RE.match(repr(vc))
    assert m, repr(vc)
    s = m.group(1).strip()
    return [int(x) for x in s.split(",")] if s else []


def _split_excess_waits(tc, ordered_instructions_by_block, max_waits=1):
    nc = tc.nc
    for _bb, insts in ordered_instructions_by_block.items():
        out = []
        for inst in insts:
            si = inst.sync_info
            waits = list(si.on_wait) if si and si.on_wait else []
            if len(waits) > max_waits:
                keep = waits[:max_waits]
                for w in waits[max_waits:]:
                    nop = mybir.InstNoOp(
                        name=nc.get_next_instruction_name(), ins=[], outs=[]
                    )
                    nop.engine = inst.engine
                    nop.sync_info = bass_rust.SyncInfo(on_wait=[w], on_update=[])
                    nc.register_instruction(nop)
                    out.append(nop)
                inst.sync_info = bass_rust.SyncInfo(
                    on_wait=keep, on_update=list(si.on_update or [])
                )
            out.append(inst)
        insts[:] = out


_orig_lower = tile.TileContext._lower_ordered_insts


def _lower_with_split(self, postordered_blocks):
    _split_excess_waits(self, postordered_blocks)
    return _orig_lower(self, postordered_blocks)


def _drain_and_barrier_split(self, tick_clock, wait_clock):
    vals = _vc_values(tick_clock.global_clock)
    for proc_idx, tick in enumerate(vals):
        if tick <= 0:
            continue
        single = [0] * len(vals)
        single[proc_idx] = tick
        nop_inst = self.nc.sync.nop(nofuse=True, hint=f"drain_wait_p{proc_idx}")
        wait_clock.add_sem_waits(
            nop_inst.ins, ScopedClock({None: bass_rust.VectorClock(single)})
        )
    self.nc.sync.drain()
    self.nc.all_engine_barrier()
    assert self.sems is not None
    popped = self.nc._tile_sem_poison_stack.pop()
    assert popped is self._sem_poison
    self.nc.clear_and_free_semaphores(list(self.sems.allocated().values()))
    self.nc.all_engine_barrier()


tile.TileContext._lower_ordered_insts = _lower_with_split
tile.TileContext._drain_and_barrier = _drain_and_barrier_split

# ---------------------------------------------------------------------------
# Problem constants (hardcoded per contract)
# ---------------------------------------------------------------------------
B = 2
SQ = 1024
SKV = 4096
HID = 1024
HEADS = 16
DH = 64
DOUT = 4096
NCORES = 8
HG = 4  # head-groups (cores per batch)
GHEADS = HEADS // HG  # heads per group = 4
GF = GHEADS * DH  # feature slice per group = 256
NPAIR = GHEADS // 2  # head pairs per group = 2

F32 = mybir.dt.float32
BF16 = mybir.dt.bfloat16
FP16 = mybir.dt.float16
DT_MM = BF16
NP_MM = "bfloat16"
Exp = mybir.ActivationFunctionType.Exp
Ln = mybir.ActivationFunctionType.Ln
MUL = mybir.AluOpType.mult
ADD = mybir.AluOpType.add

NKVT = SKV // 128  # 32 kv tiles
NKVB = 8  # kv blocks (512 wide)
NFT_Q = 4096 // 128  # 32 contraction tiles for Q proj
NFT_KV = HID // 128  # 8 contraction tiles for K/V proj
NSQH = SQ // 512  # 2 sq halves
NJT = DOUT // 128  # 32 output row tiles

_NC_CACHE = {}


def build():
    if "nc" in _NC_CACHE:
        return _NC_CACHE["nc"]
    nc = bass.Bass()

    fft = nc.declare_dram_parameter("fft", [4096, SQ], DT_MM, isOutput=False)
    kvt = nc.declare_dram_parameter("kvt", [NKVB, HID, 512], DT_MM, isOutput=False)
    wqt = nc.declare_dram_parameter("wqt", [4096, GF], DT_MM, isOutput=False)
    wkt = nc.declare_dram_parameter("wkt", [HID, GF], DT_MM, isOutput=False)
    wvt = nc.declare_dram_parameter("wvt", [HID, GF], DT_MM, isOutput=False)
    wot = nc.declare_dram_parameter("wot", [GF, DOUT], DT_MM, isOutput=False)
    bq = nc.declare_dram_parameter("bq", [128, NPAIR], F32, isOutput=False)
    bk = nc.declare_dram_parameter("bk", [128, NPAIR], F32, isOutput=False)
    outp = nc.declare_dram_parameter("outp", [NPAIR, DOUT, SQ], FP16, isOutput=True)

    with tile.TileContext(nc) as tc:
        with (
            tc.tile_pool(name="hold", bufs=1) as hold,
            tc.tile_pool(name="misc", bufs=1) as misc,
            tc.tile_pool(name="kvs", bufs=NKVB) as kvs,
        ):
            # ---- long-lived tiles ----
            wkt_r = hold.tile([128, NFT_KV, NPAIR, 128], DT_MM, tag="wkt")
            nc.sync.dma_start(
                out=wkt_r[:],
                in_=wkt.rearrange("(ft p) (pr d) -> p ft pr d", p=128, pr=NPAIR),
            )
            wvt_r = hold.tile([128, NFT_KV, GF], DT_MM, tag="wvt")
            nc.sync.dma_start(
                out=wvt_r[:], in_=wvt.rearrange("(ft p) d -> p ft d", p=128)
            )
            wot_r = hold.tile([128, NPAIR, DOUT], DT_MM, tag="wot")
            bq_t = misc.tile([128, NPAIR], F32, tag="bq")
            nc.sync.dma_start(out=bq_t[:], in_=bq[:])
            bk_t = misc.tile([128, NPAIR], F32, tag="bk")
            nc.sync.dma_start(out=bk_t[:], in_=bk[:])

            ones_f = misc.tile([128, GHEADS], F32, tag="ones_f")
            nc.vector.memset(ones_f[:], 1.0)
            ones_row = misc.tile([1, DH], DT_MM, tag="ones_row")
            nc.vector.tensor_copy(ones_row[:], ones_f[0:1, 0:1].broadcast_to([1, DH]))

            qt_r = hold.tile([128, GHEADS, SQ], DT_MM, tag="qt")
            kt_r = hold.tile([128, GHEADS, SKV], DT_MM, tag="kt")
            v_r = hold.tile([128, NKVT, GHEADS, 128], DT_MM, tag="v")
            att_r = hold.tile([128, NPAIR, SQ], DT_MM, tag="att")

            kv_blocks = [None] * NKVB

            def v_chunk(kvt_i, pool, tag):
                """Project V for one 128-wide kv tile (one kl chunk)."""
                kb, kl = divmod(kvt_i, 4)
                kv_t = kv_blocks[kb]
                v_ps = pool.tile([128, GF], F32, tag=tag,
                                 name=f"v_ps{kvt_i}")
                for ft in range(NFT_KV):
                    nc.tensor.matmul(
                        v_ps[:],
                        kv_t[:, ft, 128 * kl : 128 * (kl + 1)],
                        wvt_r[:, ft, :],
                        start=(ft == 0),
                        stop=(ft == NFT_KV - 1),
                    )
                nc.vector.tensor_copy(
                    v_r[:, kvt_i, :, 0:DH],
                    v_ps.rearrange("p (h d) -> p h d", h=GHEADS),
                )
                nc.vector.tensor_copy(v_r[:, kvt_i, :, DH : DH + 1], ones_f[:, :])

            # ================= Phase 1: Q + K projections =================
            with (
                nc.named_scope("phaseAB_proj"),
                tc.tile_pool(name="ffts", bufs=3) as ffts,
                tc.tile_pool(name="wqs", bufs=3) as wqs,
                tc.tile_pool(name="psA", bufs=4, space="PSUM") as psA,
                tc.tile_pool(name="psB", bufs=4, space="PSUM") as psB,
            ):
                qt_ps = [
                    psA.tile([128, 512], F32, tag="psA", name=f"qt_ps{i}")
                    for i in range(4)
                ]  # (pair, sqh)
                cur = {}

                def a_step(kt):
                    if kt % 2 == 0:
                        j = kt // 2
                        fft_t = ffts.tile([128, 2, SQ], DT_MM, tag="fft",
                                          name=f"fft{j}")
                        nc.scalar.dma_start(
                            out=fft_t[:],
                            in_=fft[256 * j : 256 * (j + 1), :].rearrange(
                                "(two p) s -> p two s", two=2
                            ),
                        )
                        wq_t = wqs.tile([128, 2, NPAIR, 128], DT_MM, tag="wq",
                                        name=f"wq{j}")
                        nc.sync.dma_start(
                            out=wq_t[:],
                            in_=wqt[256 * j : 256 * (j + 1), :].rearrange(
                                "(two p) (pr d) -> p two pr d", two=2, pr=NPAIR
                            ),
                        )
                        cur["fft"], cur["wq"] = fft_t, wq_t
                    half = kt % 2
                    for pr in range(NPAIR):
                        for sh in range(NSQH):
                            nc.tensor.matmul(
                                qt_ps[pr * NSQH + sh][:],
                                cur["wq"][:, half, pr, :],
                                cur["fft"][:, half, 512 * sh : 512 * (sh + 1)],
                                start=(kt == 0),
                                stop=(kt == NFT_Q - 1),
                            )

                def k_block(kb):
                    kv_t = kvs.tile([128, NFT_KV, 512], DT_MM, tag="kv",
                                    name=f"kv{kb}")
                    kv_blocks[kb] = kv_t
                    # two half-DMAs so K proj can start on ft 0-3 early
                    src = kvt[kb].rearrange("(ft p) n -> p ft n", p=128)
                    nc.gpsimd.dma_start(out=kv_t[:, 0:4, :], in_=src[:, 0:4, :])
                    nc.gpsimd.dma_start(out=kv_t[:, 4:8, :], in_=src[:, 4:8, :])
                    kb_sl = slice(512 * kb, 512 * (kb + 1))
                    for pr in range(NPAIR):
                        kt_ps = psB.tile([128, 512], F32, tag="psB",
                                         name=f"kt_ps{kb}_{pr}")
                        for ft in range(NFT_KV):
                            nc.tensor.matmul(
                                kt_ps[:],
                                wkt_r[:, ft, pr, :],
                                kv_t[:, ft, :],
                                start=(ft == 0),
                                stop=(ft == NFT_KV - 1),
                            )
                        nc.vector.tensor_scalar(
                            kt_r[0:64, 2 * pr, kb_sl],
                            kt_ps[0:64, :],
                            bk_t[0:64, pr : pr + 1],
                            None,
                            ADD,
                        )
                        nc.vector.tensor_scalar(
                            kt_r[64:128, 2 * pr + 1, kb_sl],
                            kt_ps[64:128, :],
                            bk_t[64:128, pr : pr + 1],
                            None,
                            ADD,
                        )
                        nc.gpsimd.dma_start(
                            out=kt_r[64:128, 2 * pr, kb_sl],
                            in_=kt_r[0:64, 2 * pr, kb_sl],
                        )
                        nc.gpsimd.dma_start(
                            out=kt_r[0:64, 2 * pr + 1, kb_sl],
                            in_=kt_r[64:128, 2 * pr + 1, kb_sl],
                        )

                for kt in range(NFT_Q):
                    a_step(kt)
                    if kt % 4 == 0:
                        k_block(kt // 4)
                    if kt >= 28:
                        v_chunk(kt - 28, psB, "psB")  # kv block 0

                # activation-table warm: force the natural_log_exp set to
                # load now (ACT idle; all phase-AB DMAs already issued).
                warm = misc.tile([1, 64], F32, tag="warm")
                nc.vector.memset(warm[:], 1.0)
                warm2 = misc.tile([1, 64], F32, tag="warm2")
                nc.scalar.activation(warm2[:], warm[:], Ln)
                nc.scalar.activation(warm2[:], warm2[:], Exp)

                for pr in range(NPAIR):
                    for sh in range(NSQH):
                        sq_sl = slice(512 * sh, 512 * (sh + 1))
                        nc.vector.tensor_scalar(
                            qt_r[0:64, 2 * pr, sq_sl],
                            qt_ps[pr * NSQH + sh][0:64, :],
                            bq_t[0:64, pr : pr + 1],
                            None,
                            ADD,
                        )
                        nc.vector.tensor_scalar(
                            qt_r[64:128, 2 * pr + 1, sq_sl],
                            qt_ps[pr * NSQH + sh][64:128, :],
                            bq_t[64:128, pr : pr + 1],
                            None,
                            ADD,
                        )
                # duplicate halves so score matmuls contract K=128 (2x scores,
                # folded into the exp scale) -- keeps the PE fully row-active.
                for pr in range(NPAIR):
                    nc.gpsimd.dma_start(
                        out=qt_r[64:128, 2 * pr, :], in_=qt_r[0:64, 2 * pr, :]
                    )
                    nc.gpsimd.dma_start(
                        out=qt_r[0:64, 2 * pr + 1, :],
                        in_=qt_r[64:128, 2 * pr + 1, :],
                    )

            # ================= Phase C: attention =================
            # wot is only needed mid-phase-C; its DMA overlaps the start.
            nc.sync.dma_start(
                out=wot_r[:], in_=wot.rearrange("(pr p) j -> p pr j", p=128)
            )
            with (
                nc.named_scope("phaseC_attn"),
                tc.tile_pool(name="pp", bufs=3) as pp,
                tc.tile_pool(name="nrm", bufs=2) as nrm,
                tc.tile_pool(name="osb", bufs=2) as osb,
                tc.tile_pool(name="psS", bufs=2, space="PSUM") as psS,
                tc.tile_pool(name="psAtt", bufs=2, space="PSUM") as psAtt,
                tc.tile_pool(name="psX", bufs=2, space="PSUM") as psX,
            ):
                def emit_rec(hd, den):
                    # reciprocal of the softmax denominator on ACT:
                    # rec = exp(-ln(den)); same activation table set as exp.
                    rec = nrm.tile([1, NSQH, 512], DT_MM, tag="rec",
                                   name=f"rec{hd}")
                    for sh in range(NSQH):
                        t1 = nrm.tile([1, 512], F32, tag="lnd",
                                      name=f"lnd{hd}{sh}")
                        nc.scalar.activation(t1[:], den[:, sh, :], Ln)
                        with nc.allow_low_precision(reason="softmax recip"):
                            nc.scalar.activation(rec[:, sh, :], t1[:], Exp,
                                                 scale=-1.0)
                    return rec

                def emit_norm(hd, att_sb, rec, sh):
                    pr, h = hd // 2, hd % 2
                    sq_sl = slice(512 * sh, 512 * (sh + 1))
                    rb = psX.tile([DH, 512], F32, tag="psX", name=f"rb{hd}{sh}")
                    nc.tensor.matmul(
                        rb[:], ones_row[0:1, :], rec[0:1, sh, :],
                        start=True, stop=True,
                    )
                    with nc.allow_low_precision(reason="bf16 att store"):
                        nc.vector.tensor_tensor(
                            att_r[64 * h : 64 * (h + 1), pr, sq_sl],
                            att_sb[:, sh, :],
                            rb[:],
                            MUL,
                        )

                def _copy_osb(o_sb, sh, src, eng):
                    dst = o_sb[:, 512 * sh : 512 * (sh + 1)]
                    if eng == "scalar":
                        nc.scalar.activation(
                            dst, src, mybir.ActivationFunctionType.Copy
                        )
                    elif eng == "gpsimd":
                        nc.gpsimd.tensor_copy(dst, src)
                    else:
                        nc.vector.tensor_copy(dst, src)

                def d_jt(jt, pr, pool, tag, shape2, cp_engines):
                    j_sl = slice(128 * jt, 128 * (jt + 1))
                    o_sb = osb.tile([128, SQ], FP16, tag="osb",
                                    name=f"osb{pr}_{jt}")
                    if shape2:
                        o_ps = pool.tile([128, NSQH, 512], F32, tag=tag,
                                         name=f"o{pr}_{jt}")
                        for sh in range(NSQH):
                            nc.tensor.matmul(
                                o_ps[:, sh, :],
                                wot_r[:, pr, j_sl],
                                att_r[:, pr, 512 * sh : 512 * (sh + 1)],
                                start=True,
                                stop=True,
                            )
                        for sh in range(NSQH):
                            _copy_osb(o_sb, sh, o_ps[:, sh, :],
                                      cp_engines[sh % len(cp_engines)])
                    else:
                        for sh in range(NSQH):
                            o_ps = pool.tile([128, 512], F32, tag=tag,
                                             name=f"o{pr}_{jt}_{sh}")
                            nc.tensor.matmul(
                                o_ps[:],
                                wot_r[:, pr, j_sl],
                                att_r[:, pr, 512 * sh : 512 * (sh + 1)],
                                start=True,
                                stop=True,
                            )
                            _copy_osb(o_sb, sh, o_ps[:],
                                      cp_engines[sh % len(cp_engines)])
                    nc.sync.dma_start(out=outp[pr, j_sl, :], in_=o_sb[:])

                # D-pr0 slots: (head, kv) pairs during heads 2-3
                d0_slots = [(2, kv) for kv in range(6, 32)] + [
                    (3, kv) for kv in range(0, 6)
                ]
                d0_iter = iter(range(NJT))
                d0_map = {slot: jt for slot, jt in zip(d0_slots, d0_iter)}

                pending = []  # (hd, att_sb, den) awaiting norm emission

                for hd in range(GHEADS):
                    att_ps = [
                        psAtt.tile([128, 512], F32, tag="att",
                                   name=f"att{hd}_{sh}")
                        for sh in range(NSQH)
                    ]

                    def pv(kv, p, att_ps=att_ps, hd=hd):
                        for sh in range(NSQH):
                            nc.tensor.matmul(
                                att_ps[sh][:],
                                v_r[:, kv, hd, :],
                                p[:, sh, :],
                                start=(kv == 0),
                                stop=(kv == NKVT - 1),
                            )

                    pq = []  # pending (kv, p) awaiting PV
                    for kv in range(NKVT):
                        s_ps = psS.tile([128, NSQH, 512], F32, tag="s",
                                        name=f"s{hd}_{kv}")
                        kv_sl = slice(128 * kv, 128 * (kv + 1))
                        for sh in range(NSQH):
                            nc.tensor.matmul(
                                s_ps[:, sh, :],
                                kt_r[:, hd, kv_sl],
                                qt_r[:, hd, 512 * sh : 512 * (sh + 1)],
                                start=True,
                                stop=True,
                            )
                        p = pp.tile([128, NSQH, 512], DT_MM, tag="p",
                                    name=f"p{hd}_{kv}")
                        nc.scalar.activation(p[:], s_ps[:], Exp, scale=0.0625)
                        pq.append((kv, p))
                        if kv >= 1:
                            pv(*pq.pop(0))
                        # ---- interleaved extra PE work ----
                        if hd == 0 and kv <= 27:
                            v_chunk(kv + 4, psX, "psX")
                        if pending and kv in (2, 4):
                            ph, psb, prec = pending[0]
                            emit_norm(ph, psb, prec, 0 if kv == 2 else 1)
                            if kv == 4:
                                pending.pop(0)
                        jt = d0_map.get((hd, kv))
                        if jt is not None:
                            d_jt(jt, 0, psX, "psX", False, ("vector",))
                    pv(*pq.pop(0))

                    # free att psum banks: copy numerator + denominator to SBUF
                    att_sb = nrm.tile([DH, NSQH, 512], F32, tag="attsb",
                                      name=f"attsb{hd}")
                    den = nrm.tile([1, NSQH, 512], F32, tag="den",
                                   name=f"den{hd}")
                    for sh in range(NSQH):
                        nc.vector.tensor_copy(att_sb[:, sh, :],
                                              att_ps[sh][0:DH, :])
                        nc.vector.tensor_copy(den[:, sh, :],
                                              att_ps[sh][DH : DH + 1, :])
                    rec = emit_rec(hd, den)
                    if hd < GHEADS - 1:
                        pending.append((hd, att_sb, rec))
                    else:
                        for sh in range(NSQH):
                            emit_norm(hd, att_sb, rec, sh)

                # ---- tail: D partial for pair 1 (deep 6-slot pipeline) ----
                pools3 = ((psX, "psX"), (psAtt, "att"), (psS, "s"))
                o_sb2 = None
                k = 0
                for jt in range(NJT):
                    if jt % 2 == 0:
                        o_sb2 = osb.tile([128, 2, SQ], FP16, tag="osb2",
                                         name=f"osb1_{jt}")
                    j_sl = slice(128 * jt, 128 * (jt + 1))
                    for sh in range(NSQH):
                        pool, tag = pools3[k % 3]
                        o_ps = pool.tile([128, 512], F32, tag=tag,
                                         name=f"o1_{jt}_{sh}")
                        nc.tensor.matmul(
                            o_ps[:],
                            wot_r[:, 1, j_sl],
                            att_r[:, 1, 512 * sh : 512 * (sh + 1)],
                            start=True,
                            stop=True,
                        )
                        dst = o_sb2[:, jt % 2, 512 * sh : 512 * (sh + 1)]
                        if k % 2 == 0:
                            nc.vector.tensor_copy(dst, o_ps[:])
                        else:
                            nc.scalar.activation(
                                dst, o_ps[:],
                                mybir.ActivationFunctionType.Copy,
                            )
                        k += 1
                    if jt % 2 == 1:
                        jp = jt // 2
                        nc.sync.dma_start(
                            out=outp[1, 256 * jp : 256 * (jp + 1), :].rearrange(
                                "(two p) s -> p two s", two=2
                            ),
                            in_=o_sb2[:],
                        )

    _NC_CACHE["nc"] = nc
    return nc


def _make_in_maps(inputs):
    ff = np.asarray(inputs["fused_features"], dtype=np.float32)
    kv_in = np.concatenate(
        [
            np.asarray(inputs["text"], dtype=np.float32),
            np.asarray(inputs["image"], dtype=np.float32),
            np.asarray(inputs["audio"], dtype=np.float32),
            np.asarray(inputs["video"], dtype=np.float32),
        ],
        axis=1,
    )
    Wq = np.asarray(inputs["Wq"], dtype=np.float32)
    Wk = np.asarray(inputs["Wk"], dtype=np.float32)
    Wv = np.asarray(inputs["Wv"], dtype=np.float32)
    Wo = np.asarray(inputs["Wo"], dtype=np.float32)
    bq = np.asarray(inputs["bq"], dtype=np.float32)
    bk = np.asarray(inputs["bk"], dtype=np.float32)

    import ml_dtypes

    np_mm = np.dtype(ml_dtypes.bfloat16) if NP_MM == "bfloat16" else np.float32
    ffT = [np.ascontiguousarray(ff[b].T.astype(np_mm)) for b in range(B)]
    kvT = [
        np.ascontiguousarray(
            kv_in[b].T.astype(np_mm).reshape(HID, NKVB, 512).transpose(1, 0, 2)
        )
        for b in range(B)
    ]
    WqT = np.ascontiguousarray(Wq.T.astype(np_mm))  # [4096, 1024]
    WkT = np.ascontiguousarray(Wk.T.astype(np_mm))  # [1024, 1024]
    WvT = np.ascontiguousarray(Wv.T.astype(np_mm))
    WoT = np.ascontiguousarray(Wo.T.astype(np_mm))  # [1024, 4096]

    in_maps = []
    for c in range(NCORES):
        b, hg = divmod(c, HG)
        fs = slice(GF * hg, GF * (hg + 1))
        in_maps.append(
            {
                "fft": ffT[b],
                "kvt": kvT[b],
                "wqt": np.ascontiguousarray(WqT[:, fs]),
                "wkt": np.ascontiguousarray(WkT[:, fs]),
                "wvt": np.ascontiguousarray(WvT[:, fs]),
                "wot": np.ascontiguousarray(WoT[fs, :]),
                "bq": np.ascontiguousarray(bq[fs].reshape(NPAIR, 128).T),
                "bk": np.ascontiguousarray(bk[fs].reshape(NPAIR, 128).T),
            }
        )
    return in_maps


def _assemble(results, bo, Wo, bv):
    out = np.zeros((B, SQ, DOUT), dtype=np.float32)
    for c in range(NCORES):
        b = c // HG
        partial = results[c]["outp"].astype(np.float32)  # [NPAIR, DOUT, SQ]
        out[b] += (partial[0] + partial[1]).T
    bo_prime = np.asarray(bo, dtype=np.float32) + np.asarray(
        Wo, dtype=np.float32
    ) @ np.asarray(bv, dtype=np.float32)
    out += bo_prime
    return out


def run_spmd(inputs, trace=False):
    nc = build()
    in_maps = _make_in_maps(inputs)
    r = run_bass_kernel_spmd(nc, in_maps, list(range(NCORES)), trace=trace)
    return _assemble(r.results, inputs["bo"], inputs["Wo"], inputs["bv"]), r


def kernel(**inputs) -> np.ndarray:
    out, _ = run_spmd(inputs, trace=False)
    return out
